# revision 5
# baseline (speedup 1.0000x reference)
"""Trainium2 Bass kernel for the N^3 triplet descriptor (gnn_message_passing).

v3: transposed layout + TensorE moment reductions + all-bf16 products.

The reference's O(N^3) angular sum factorizes exactly via the Legendre
addition theorem into O(N^2) per-pair vector moments (see host_combine).
Each device owns 96 j-neighbors (partitions) x 48 central atoms i (free
axis); the per-core moment sums over j are PARTITION reductions done on
the otherwise-idle TensorE as ones-vector matmuls (bf16 rhs, fp32 PSUM
accumulate) instead of DVE free-axis reduces.

  DVE  : min-image wrap (mask ops), r2, clamped deg-4 Chebyshev fc poly
         (w = min(r2, RC^2); the poly has an exact fp32 root at w=RC^2 so
         no cutoff mask is needed), e-family e_k = fc*r^(k-2) (bf16 out),
         27 S1/S2 pair-product blocks in all-bf16 (2x DVE perf mode), and
         the final cross-quadrant PSUM->SBUF row copy.
  ACT  : rinv = 1/sqrt(r2+eps) and rinv2 = rinv^2 (one table), bf16
         staging copies of dx/sq/poff, bulk PSUM->SBUF copy.
  Pool : constant memsets + off-diagonal dx_a*dx_b products.
  PE   : 2 warm-up matmuls (cold p-state), then 4 ones-matmul reductions
         into PSUM partition rows 0/32/64 (bank A) + 0 (bank B).
Output: the four moment rows land on SBUF partitions 0/32/64/96 of one
tile, leaving as ONE 4-descriptor DMA.  Input DMA queues are warmed by a
1-descriptor dummy so the 96-descriptor replicated input launches ~0.5us
earlier.  The tiny nonlinear combine runs on host in fp64.
"""

import numpy as np

import concourse.bass as bass
import concourse.bacc as bacc
from concourse import mybir
from concourse.bass_utils import run_bass_kernel_spmd

F32 = mybir.dt.float32
BF16 = mybir.dt.bfloat16
ALU = mybir.AluOpType
ACT = mybir.ActivationFunctionType

N = 192
NJ = 96          # j neighbors per core (partition dim)
NI = 48          # central atoms per core (free dim)
NJB = 2          # j halves
NIC = 4          # i chunks
BOX_L = 20.0
RC = 5.0
FC_DEG = 4       # deg-4 fit err 4.2e-5; feature-level impact ~1e-3 rel
R2_EPS = 1e-12

_FC_W = np.linspace(0, RC * RC, 20001)
_FC_Y = 0.5 * (1 + np.cos(np.pi * np.sqrt(_FC_W) / RC))
_FC_C = (
    np.polynomial.chebyshev.Chebyshev.fit(_FC_W, _FC_Y, FC_DEG, domain=[0, RC * RC])
    .convert(kind=np.polynomial.Polynomial)
    .coef.astype(np.float64)
)


def _horner_tail_f32(w):
    f = np.float32
    yh = f(f(w) * f(_FC_C[FC_DEG]))
    for k in range(FC_DEG - 1, 0, -1):
        yh = f(f(f(_FC_C[k]) + yh) * f(w))
    return yh


# fc(w) = C0P + yh(w); C0P chosen so fc(RC^2) == 0 exactly in fp32
C0P = float(-_horner_tail_f32(RC * RC))
try:
    import ml_dtypes
    C0SELF = float(np.float32(C0P).astype(ml_dtypes.bfloat16).astype(np.float32))
except Exception:
    C0SELF = C0P

_cached = {}


def _v(ap, off, dims):
    return bass.AP(ap.tensor, ap.offset + off, [list(ap.ap[0])] + [list(d) for d in dims])


def build_nc():
    _orig_barrier = bass.Bass.all_engine_barrier
    _orig_memset = bass.BassSharedVectorInterface.memset
    bass.Bass.all_engine_barrier = lambda self: None
    bass.BassSharedVectorInterface.memset = lambda self, ap, v: None
    try:
        nc = bacc.Bacc(
            "TRN2",
            target_bir_lowering=False,
            debug=False,
            enable_asserts=True,
            num_devices=NJB * NIC,
        )
    finally:
        bass.Bass.all_engine_barrier = _orig_barrier
        bass.BassSharedVectorInterface.memset = _orig_memset

    rji_d = nc.dram_tensor("rji", [NJ, 160], F32, kind="ExternalInput").ap()
    out_d = nc.dram_tensor("out", [4, 432], F32, kind="ExternalOutput").ap()

    rji = nc.alloc_sbuf_tensor("rji_s", [NJ, 160], F32).ap()
    dxr = nc.alloc_sbuf_tensor("dxr", [NJ, 144], F32).ap()
    scr = nc.alloc_sbuf_tensor("scr", [NJ, 288], F32).ap()     # wrap scratch
    geo = nc.alloc_sbuf_tensor("geo", [NJ, 288], F32).ap()     # dx | sq
    poff = nc.alloc_sbuf_tensor("poff", [NJ, 144], F32).ap()
    geobf = nc.alloc_sbuf_tensor("geobf", [NJ, 432], BF16).ap()  # dx|sq|poff bf16
    r2w = nc.alloc_sbuf_tensor("r2w", [NJ, 96], F32).ap()      # r2 | w
    yh = nc.alloc_sbuf_tensor("yh", [NJ, NI], F32).ap()
    rvp = nc.alloc_sbuf_tensor("rvp", [NJ, 144], F32).ap()     # rinv2 | rinv | one
    wx = nc.alloc_sbuf_tensor("wx", [NJ, 11 * NI], BF16).ap()  # e0..e10
    big = nc.alloc_sbuf_tensor("big", [NJ, 27 * NI], BF16).ap()
    outsb = nc.alloc_sbuf_tensor("outsb", [97, 432], F32).ap()
    ones_bf = nc.alloc_sbuf_tensor("ones_bf", [NJ, 1], BF16).ap()
    scra = nc.alloc_sbuf_tensor("scra", [1, 8], F32).ap()
    c_eps = nc.alloc_sbuf_tensor("c_eps", [128, 1], F32).ap()
    nc.const_aps.aps[(F32, R2_EPS)] = c_eps

    psum_out = nc.alloc_psum_tensor("psum_out", [65, 432], F32).ap()
    psum_out2 = nc.alloc_psum_tensor("psum_out2", [1, 432], F32).ap()
    psum_warm = nc.alloc_psum_tensor("psum_warm", [1, 2], F32).ap()

    dsem = nc.alloc_semaphore("dsem")
    vq = nc.alloc_semaphore("vq")
    sqm = nc.alloc_semaphore("sqm")
    gq = nc.alloc_semaphore("gq")
    mmq = nc.alloc_semaphore("mmq")

    dx = geo[:, 0:144]
    sq_t = geo[:, 144:288]
    r2 = r2w[:, 0:NI]
    w = r2w[:, NI:2 * NI]

    rj3 = rji[:, 0:144].rearrange("p (d j) -> p d j", d=3)
    ri3 = rji[:, 144:147].unsqueeze(-1).broadcast_to((NJ, 3, NI))
    dxr3 = dxr.rearrange("p (d j) -> p d j", d=3)

    c = [float(x) for x in _FC_C]

    VQ_DX = 5
    VQ_SQ = 6
    VQ_R2 = 7
    VQ_W = 8
    VQ_ECH = 16
    VQ_S1P = 17
    VQ_S2DP = 18
    VQ_ALL = 19
    VQ_CPB = 20
    GQ_EPS = 2
    GQ_POFF = 5
    GQ_W2 = 6
    MM_WARM = 2
    MM_CPA = 5                 # radial + S1 + S2d matmuls done
    MM_ALL = 6
    SQ_GEOBF = 3               # dummy, dxbf, sqbf
    SQ_RINV2 = 5
    SQ_POFFBF = 6
    SQ_COPY = 7

    with nc.Block(no_gpsimd_drain=True) as block:

        @block.sync
        def _(sync):
            # both input halves on the sync HWDGE: the scalar engine's queue
            # is blocked ~2.6us by the ACT table loads at block start
            sync.dma_start(rji[:, 0:80], rji_d[:, 0:80]).then_inc(dsem, 16)
            sync.dma_start(rji[:, 80:160], rji_d[:, 80:160]).then_inc(dsem, 16)
            sync.wait_ge(sqm, SQ_COPY)
            sync.wait_ge(vq, VQ_CPB)
            sync.dma_start(
                out_d,
                bass.AP(outsb.tensor, 0, [[32 * 432, 4], [1, 432]]),
            ).then_inc(dsem, 16)
            sync.wait_ge(dsem, 48)

        @block.tensor
        def _(tensor):
            tn = [0]

            def T(inst):
                # no intra-PE serialization: MMs are independent (disjoint
                # PSUM) and the PE is strict FIFO; keep only the counter
                inst.then_inc(mmq, 1)
                tn[0] += 1
                return inst

            tensor.wait_ge(gq, 1)
            T(tensor.matmul(psum_warm[0:1, 0:1], ones_bf, ones_bf))
            T(tensor.matmul(psum_warm[0:1, 1:2], ones_bf, ones_bf))
            assert tn[0] == MM_WARM
            # p-state keep-warm pings while the DVE streams geometry/fc
            tensor.wait_ge(vq, VQ_R2)
            tensor.matmul(psum_warm[0:1, 0:1], ones_bf, ones_bf)
            tensor.wait_ge(vq, 13)
            tensor.matmul(psum_warm[0:1, 1:2], ones_bf, ones_bf)
            tensor.wait_ge(vq, VQ_ECH)
            T(tensor.matmul(psum_out[0:1, :], ones_bf, wx[:, 2 * NI:11 * NI]))
            tensor.wait_ge(vq, VQ_S1P)
            T(tensor.matmul(psum_out[32:33, :], ones_bf, big[:, 0:9 * NI]))
            tensor.wait_ge(vq, VQ_S2DP)
            T(tensor.matmul(psum_out[64:65, :], ones_bf, big[:, 9 * NI:18 * NI]))
            assert tn[0] == MM_CPA
            tensor.wait_ge(vq, VQ_ALL)
            T(tensor.matmul(psum_out2[0:1, :], ones_bf, big[:, 18 * NI:27 * NI]))
            assert tn[0] == MM_ALL

        @block.scalar
        def _(scalar):
            sn = [0]

            def S(inst):
                if sn[0] > 0:
                    inst._wait_ge(sqm, sn[0])
                inst.then_inc(sqm, 1)
                sn[0] += 1
                return inst

            scalar.wait_ge(gq, GQ_EPS)
            S(scalar.activation(
                scra[0:1, 0:1], c_eps[0:1, :], ACT.Abs_reciprocal_sqrt,
                bias=R2_EPS))
            # bf16 staging copies for the 2x-mode products
            scalar.wait_ge(vq, VQ_DX)
            S(scalar.activation(geobf[:, 0:144], dx, ACT.Copy))
            scalar.wait_ge(vq, VQ_SQ)
            S(scalar.activation(geobf[:, 144:288], sq_t, ACT.Copy))
            scalar.wait_ge(vq, VQ_R2)
            S(scalar.activation(rvp[:, NI:2 * NI], r2, ACT.Abs_reciprocal_sqrt,
                                bias=R2_EPS))
            S(scalar.activation(rvp[:, 0:NI], rvp[:, NI:2 * NI], ACT.Square,
                                bias=R2_EPS))
            assert sn[0] == SQ_RINV2
            scalar.wait_ge(gq, GQ_POFF)
            S(scalar.activation(geobf[:, 288:432], poff, ACT.Copy))
            assert sn[0] == SQ_POFFBF
            scalar.wait_ge(mmq, MM_CPA)
            S(scalar.activation(outsb[0:65, :], psum_out, ACT.Copy))
            assert sn[0] == SQ_COPY

        @block.gpsimd
        def _(gpsimd):
            gn = [0]

            def G(inst):
                if gn[0] > 0:
                    inst._wait_ge(gq, gn[0])
                inst.then_inc(gq, 1)
                gn[0] += 1
                return inst

            G(gpsimd.memset(ones_bf, 1.0))
            G(gpsimd.memset(c_eps, R2_EPS))
            G(gpsimd.memset(rvp[:, 2 * NI:3 * NI], 1.0))
            gpsimd.wait_ge(vq, VQ_DX)
            G(gpsimd.tensor_tensor(
                poff[:, 0:96], dx[:, 0:96], dx[:, 48:144], op=ALU.mult))
            G(gpsimd.tensor_tensor(
                poff[:, 96:144], dx[:, 0:NI], dx[:, 96:144], op=ALU.mult))
            assert gn[0] == GQ_POFF
            # w^2 for the e-chain (into the freed wrap scratch)
            gpsimd.wait_ge(vq, VQ_W)
            G(gpsimd.tensor_tensor(
                scr[:, 0:NI], r2w[:, NI:2 * NI], r2w[:, NI:2 * NI],
                op=ALU.mult))
            assert gn[0] == GQ_W2

        @block.vector
        def _(vector):
            vn = [0]

            def V(inst):
                if vn[0] > 0:
                    inst._wait_ge(vq, vn[0])
                inst.then_inc(vq, 1)
                vn[0] += 1
                return inst

            vector.wait_ge(dsem, 32)
            V(vector.tensor_tensor(dxr3, rj3, ri3, op=ALU.subtract))
            # minimum image (box = BOX_L * I)
            V(vector.tensor_scalar(
                scr[:, 0:144], dxr, BOX_L / 2, BOX_L, op0=ALU.is_gt, op1=ALU.mult))
            V(vector.tensor_tensor(
                scr[:, 144:288], dxr, scr[:, 0:144], op=ALU.subtract))
            V(vector.tensor_scalar(
                scr[:, 0:144], dxr, -BOX_L / 2, BOX_L, op0=ALU.is_lt, op1=ALU.mult))
            V(vector.tensor_tensor(
                dx, scr[:, 144:288], scr[:, 0:144], op=ALU.add))
            assert vn[0] == VQ_DX
            V(vector.tensor_tensor(sq_t, dx, dx, op=ALU.mult))
            assert vn[0] == VQ_SQ
            V(vector.reduce_sum(
                r2, _v(geo, 144, [[1, NI], [NI, 3]]),
                axis=mybir.AxisListType.X,
            ))
            assert vn[0] == VQ_R2
            V(vector.tensor_scalar(w, r2, RC * RC, None, op0=ALU.min))
            assert vn[0] == VQ_W
            V(vector.tensor_scalar(yh, w, c[FC_DEG], None, op0=ALU.mult))
            for k in range(FC_DEG - 1, 0, -1):
                V(vector.scalar_tensor_tensor(
                    yh, yh, c[k], w, op0=ALU.add, op1=ALU.mult))
            # e-family (bf16): [e0|e1|e2] = (C0P+yh) * [rinv2|rinv|1]
            vector.wait_ge(sqm, SQ_RINV2)
            vector.wait_ge(gq, 3)
            V(vector.scalar_tensor_tensor(
                _v(wx, 0, [[NI, 3], [1, NI]]),
                _v(yh, 0, [[0, 3], [1, NI]]),
                C0P,
                rvp[:, 0:144],
                op0=ALU.add, op1=ALU.mult))
            # [e3|e4] = [e1|e2]*w ; [e5..e8] = [e1..e4]*w2 ; [e9|e10] = [e5|e6]*w2
            V(vector.tensor_tensor(
                _v(wx, 3 * NI, [[NI, 2], [1, NI]]),
                _v(wx, NI, [[NI, 2], [1, NI]]),
                _v(r2w, NI, [[0, 2], [1, NI]]),
                op=ALU.mult))
            vector.wait_ge(gq, GQ_W2)
            V(vector.tensor_tensor(
                _v(wx, 5 * NI, [[NI, 4], [1, NI]]),
                _v(wx, NI, [[NI, 4], [1, NI]]),
                _v(scr, 0, [[0, 4], [1, NI]]),
                op=ALU.mult))
            V(vector.tensor_tensor(
                _v(wx, 9 * NI, [[NI, 2], [1, NI]]),
                _v(wx, 5 * NI, [[NI, 2], [1, NI]]),
                _v(scr, 0, [[0, 2], [1, NI]]),
                op=ALU.mult))
            assert vn[0] == VQ_ECH
            # all-bf16 S1/S2 product blocks (2x DVE mode)
            vector.wait_ge(sqm, SQ_GEOBF)
            V(vector.tensor_tensor(
                _v(big, 0, [[3 * NI, 3], [NI, 3], [1, NI]]),
                _v(wx, NI, [[NI, 3], [0, 3], [1, NI]]),
                _v(geobf, 0, [[0, 3], [NI, 3], [1, NI]]),
                op=ALU.mult))
            assert vn[0] == VQ_S1P
            V(vector.tensor_tensor(
                _v(big, 9 * NI, [[3 * NI, 3], [NI, 3], [1, NI]]),
                _v(wx, 0, [[NI, 3], [0, 3], [1, NI]]),
                _v(geobf, 144, [[0, 3], [NI, 3], [1, NI]]),
                op=ALU.mult))
            assert vn[0] == VQ_S2DP
            vector.wait_ge(sqm, SQ_POFFBF)
            V(vector.tensor_tensor(
                _v(big, 18 * NI, [[3 * NI, 3], [NI, 3], [1, NI]]),
                _v(wx, 0, [[NI, 3], [0, 3], [1, NI]]),
                _v(geobf, 288, [[0, 3], [NI, 3], [1, NI]]),
                op=ALU.mult))
            assert vn[0] == VQ_ALL
            # evacuate the bank-B moment row cross-quadrant to partition 96
            vector.wait_ge(mmq, MM_ALL)
            V(vector.tensor_scalar(
                outsb[96:97, :], psum_out2, 1.0, None, op0=ALU.mult))
            assert vn[0] == VQ_CPB

    nc.compile()
    return nc


def host_prep(R):
    """Per-core input arrays: [96, 160] = [R_i-chunk^T replicated | R_j | pad]."""
    R = np.ascontiguousarray(R, np.float32)
    in_maps = []
    for core in range(NJB * NIC):
        jh, ic = divmod(core, NIC)
        rji = np.zeros((NJ, 160), np.float32)
        ri = R[ic * NI:(ic + 1) * NI, :]              # [48, 3] central chunk
        rji[:, 0:144] = ri.T.reshape(1, 144)          # d-major, replicated
        rji[:, 144:147] = R[jh * NJ:(jh + 1) * NJ, :]
        in_maps.append({"rji": rji})
    return in_maps


def host_combine(partials):
    """partials: list of 8 [4,432] arrays (core order jh*4+ic). [192,18]."""
    sums = np.zeros((N, 36), np.float64)
    for core, p in enumerate(partials):
        jh, ic = divmod(core, NIC)
        q = p.astype(np.float64).reshape(4, 9, NI)
        sl = slice(ic * NI, (ic + 1) * NI)
        sums[sl, 0:9] += q[0].T
        sums[sl, 9:18] += q[1].T
        sums[sl, 18:27] += q[2].T
        sums[sl, 27:36] += q[3].T
    sums = sums.astype(np.float32)
    q_r = sums[:, 0:9].copy()
    q_r[:, 0] -= C0SELF
    s0 = q_r[:, 0:3]
    s1 = sums[:, 9:18].reshape(N, 3, 3)
    s2d = sums[:, 18:27].reshape(N, 3, 3)
    s2o = sums[:, 27:36].reshape(N, 3, 3)
    ang = np.empty((N, 3, 3), np.float32)
    ang[:, :, 0] = s0 * s0
    ang[:, :, 1] = (s1 * s1).sum(-1)
    fro2 = (s2d * s2d).sum(-1) + 2.0 * (s2o * s2o).sum(-1)
    ang[:, :, 2] = 1.5 * fro2 - 0.5 * s0 * s0
    return np.concatenate([q_r, ang.reshape(N, 9)], axis=-1)


def _get_nc():
    if "nc" not in _cached:
        _cached["nc"] = build_nc()
    return _cached["nc"]


def _make_runner(nc, n_cores):
    import jax
    from jax.sharding import Mesh, PartitionSpec
    from concourse import bass2jax
    from concourse import mybir as _mb

    shard_map = bass2jax.shard_map

    bass2jax.install_neuronx_cc_hook()
    partition_name = (
        nc.partition_id_tensor.name if nc.partition_id_tensor else None
    )
    in_names, out_names, out_avals = [], [], []
    for alloc in nc.m.functions[0].allocations:
        if not isinstance(alloc, _mb.MemoryLocationSet):
            continue
        name = alloc.memorylocations[0].name
        if alloc.kind == "ExternalInput":
            if name != partition_name:
                in_names.append(name)
        elif alloc.kind == "ExternalOutput":
            out_names.append(name)
            out_avals.append(jax.core.ShapedArray(
                tuple(alloc.tensor_shape), _mb.dt.np(alloc.dtype)))
    n_params = len(in_names)
    all_names = in_names + out_names
    if partition_name is not None:
        all_names = all_names + [partition_name]
    all_names = tuple(all_names)

    def _body(*args):
        operands = list(args)
        if partition_name is not None:
            operands.append(bass2jax.partition_id_tensor())
        outs = bass2jax._bass_exec_p.bind(
            *operands,
            out_avals=tuple(out_avals),
            in_names=all_names,
            out_names=tuple(out_names),
            lowering_input_output_aliases=(),
            sim_require_finite=True,
            sim_require_nnan=True,
            nc=nc,
        )
        return tuple(outs)

    devices = jax.devices()[:n_cores]
    mesh = Mesh(np.asarray(devices), ("core",))
    n_outs = len(out_names)
    sharded = jax.jit(
        shard_map(
            _body, mesh=mesh,
            in_specs=(PartitionSpec("core"),) * (n_params + n_outs),
            out_specs=(PartitionSpec("core"),) * n_outs,
            check_rep=False,
        ),
        donate_argnums=tuple(range(n_params, n_params + n_outs)),
        keep_unused=True,
    )

    def run(in_maps):
        concat_in = [
            np.concatenate([np.asarray(m[name]) for m in in_maps], axis=0)
            for name in in_names
        ]
        concat_zeros = [
            np.zeros((n_cores * a.shape[0], *a.shape[1:]), a.dtype)
            for a in out_avals
        ]
        out_arrs = sharded(*concat_in, *concat_zeros)
        return [
            {
                name: np.asarray(out_arrs[i]).reshape(
                    n_cores, *out_avals[i].shape)[c]
                for i, name in enumerate(out_names)
            }
            for c in range(n_cores)
        ]

    return run


def _get_runner():
    if "runner" not in _cached:
        _cached["runner"] = _make_runner(_get_nc(), NJB * NIC)
    return _cached["runner"]


def kernel(R, box):
    R = np.asarray(R, np.float32)
    box = np.asarray(box, np.float32)
    assert R.shape == (N, 3)
    assert np.allclose(box, np.eye(3, dtype=np.float32) * BOX_L), (
        "kernel compiled for box = 20*I"
    )
    in_maps = host_prep(R)
    results = _get_runner()(in_maps)
    partials = [results[c]["out"] for c in range(NJB * NIC)]
    return host_combine(partials)


# revision 12
# speedup vs baseline: 1.0268x; 1.0268x over previous
"""Trainium2 Bass kernel for the N^3 triplet descriptor (gnn_message_passing).

v3: transposed layout + TensorE moment reductions + all-bf16 products.

The reference's O(N^3) angular sum factorizes exactly via the Legendre
addition theorem into O(N^2) per-pair vector moments (see host_combine).
Each device owns 96 j-neighbors (partitions) x 48 central atoms i (free
axis); the per-core moment sums over j are PARTITION reductions done on
the otherwise-idle TensorE as ones-vector matmuls (bf16 rhs, fp32 PSUM
accumulate) instead of DVE free-axis reduces.

  DVE  : min-image wrap (mask ops), r2, clamped deg-4 Chebyshev fc poly
         (w = min(r2, RC^2); the poly has an exact fp32 root at w=RC^2 so
         no cutoff mask is needed), e-family e_k = fc*r^(k-2) (bf16 out),
         27 S1/S2 pair-product blocks in all-bf16 (2x DVE perf mode), and
         the final cross-quadrant PSUM->SBUF row copy.
  ACT  : rinv = 1/sqrt(r2+eps) and rinv2 = rinv^2 (one table), bf16
         staging copies of dx/sq/poff, bulk PSUM->SBUF copy.
  Pool : constant memsets + off-diagonal dx_a*dx_b products.
  PE   : 2 warm-up matmuls (cold p-state), then 4 ones-matmul reductions
         into PSUM partition rows 0/32/64 (bank A) + 0 (bank B).
Output: the four moment rows land on SBUF partitions 0/32/64/96 of one
tile, leaving as ONE 4-descriptor DMA.  Input DMA queues are warmed by a
1-descriptor dummy so the 96-descriptor replicated input launches ~0.5us
earlier.  The tiny nonlinear combine runs on host in fp64.
"""

import numpy as np

import concourse.bass as bass
import concourse.bacc as bacc
from concourse import mybir
from concourse.bass_utils import run_bass_kernel_spmd

F32 = mybir.dt.float32
BF16 = mybir.dt.bfloat16
ALU = mybir.AluOpType
ACT = mybir.ActivationFunctionType

N = 192
NJ = 96          # j neighbors per core (partition dim)
NI = 48          # central atoms per core (free dim)
NJB = 2          # j halves
NIC = 4          # i chunks
BOX_L = 20.0
RC = 5.0
FC_DEG = 4       # deg-4 fit err 4.2e-5; feature-level impact ~1e-3 rel
R2_EPS = 1e-12

_FC_W = np.linspace(0, RC * RC, 20001)
_FC_Y = 0.5 * (1 + np.cos(np.pi * np.sqrt(_FC_W) / RC))
_FC_C = (
    np.polynomial.chebyshev.Chebyshev.fit(_FC_W, _FC_Y, FC_DEG, domain=[0, RC * RC])
    .convert(kind=np.polynomial.Polynomial)
    .coef.astype(np.float64)
)


def _horner_tail_f32(w):
    f = np.float32
    yh = f(f(w) * f(_FC_C[FC_DEG]))
    for k in range(FC_DEG - 1, 0, -1):
        yh = f(f(f(_FC_C[k]) + yh) * f(w))
    return yh


# fc(w) = C0P + yh(w); C0P chosen so fc(RC^2) == 0 exactly in fp32
C0P = float(-_horner_tail_f32(RC * RC))
try:
    import ml_dtypes
    C0SELF = float(np.float32(C0P).astype(ml_dtypes.bfloat16).astype(np.float32))
except Exception:
    C0SELF = C0P

_cached = {}


def _v(ap, off, dims):
    return bass.AP(ap.tensor, ap.offset + off, [list(ap.ap[0])] + [list(d) for d in dims])


def build_nc():
    _orig_barrier = bass.Bass.all_engine_barrier
    _orig_memset = bass.BassSharedVectorInterface.memset
    bass.Bass.all_engine_barrier = lambda self: None
    bass.BassSharedVectorInterface.memset = lambda self, ap, v: None
    try:
        nc = bacc.Bacc(
            "TRN2",
            target_bir_lowering=False,
            debug=False,
            enable_asserts=True,
            num_devices=NJB * NIC,
        )
    finally:
        bass.Bass.all_engine_barrier = _orig_barrier
        bass.BassSharedVectorInterface.memset = _orig_memset

    rji_d = nc.dram_tensor("rji", [NJ, 160], F32, kind="ExternalInput").ap()
    out_d = nc.dram_tensor("out", [4, 432], F32, kind="ExternalOutput").ap()

    rji = nc.alloc_sbuf_tensor("rji_s", [NJ, 160], F32).ap()
    dxr = nc.alloc_sbuf_tensor("dxr", [NJ, 144], F32).ap()
    scr = nc.alloc_sbuf_tensor("scr", [NJ, 288], F32).ap()     # wrap scratch
    geo = nc.alloc_sbuf_tensor("geo", [NJ, 288], F32).ap()     # dx | sq
    poff = nc.alloc_sbuf_tensor("poff", [NJ, 144], F32).ap()
    geobf = nc.alloc_sbuf_tensor("geobf", [NJ, 432], BF16).ap()  # dx|sq|poff bf16
    r2w = nc.alloc_sbuf_tensor("r2w", [NJ, 96], F32).ap()      # r2 | w
    yh = nc.alloc_sbuf_tensor("yh", [NJ, NI], F32).ap()
    rvp = nc.alloc_sbuf_tensor("rvp", [NJ, 144], F32).ap()     # rinv2 | rinv | one
    wx = nc.alloc_sbuf_tensor("wx", [NJ, 11 * NI], BF16).ap()  # e0..e10
    big = nc.alloc_sbuf_tensor("big", [NJ, 27 * NI], BF16).ap()
    outsb = nc.alloc_sbuf_tensor("outsb", [97, 432], F32).ap()
    ones_bf = nc.alloc_sbuf_tensor("ones_bf", [NJ, 1], BF16).ap()
    scra = nc.alloc_sbuf_tensor("scra", [1, 8], F32).ap()
    c_eps = nc.alloc_sbuf_tensor("c_eps", [128, 1], F32).ap()
    nc.const_aps.aps[(F32, R2_EPS)] = c_eps

    psum_out = nc.alloc_psum_tensor("psum_out", [65, 432], F32).ap()
    psum_out2 = nc.alloc_psum_tensor("psum_out2", [1, 432], F32).ap()
    psum_warm = nc.alloc_psum_tensor("psum_warm", [1, 2], F32).ap()

    dsem = nc.alloc_semaphore("dsem")
    vq = nc.alloc_semaphore("vq")
    sqm = nc.alloc_semaphore("sqm")
    gq = nc.alloc_semaphore("gq")
    mmq = nc.alloc_semaphore("mmq")

    dx = geo[:, 0:144]
    sq_t = geo[:, 144:288]
    r2 = r2w[:, 0:NI]
    w = r2w[:, NI:2 * NI]

    rj3 = rji[:, 0:144].rearrange("p (d j) -> p d j", d=3)
    ri3 = rji[:, 144:147].unsqueeze(-1).broadcast_to((NJ, 3, NI))
    dxr3 = dxr.rearrange("p (d j) -> p d j", d=3)

    c = [float(x) for x in _FC_C]

    VQ_DX = 5
    VQ_SQ = 6
    VQ_R2 = 7
    VQ_W = 8
    VQ_ECH = 16
    VQ_S1P = 17
    VQ_S2DP = 18
    VQ_ALL = 19
    VQ_CPB = 20
    GQ_EPS = 2
    GQ_POFF = 5
    GQ_W2 = 6
    MM_WARM = 2
    MM_CPA = 5                 # radial + S1 + S2d matmuls done
    MM_ALL = 6
    SQ_GEOBF = 3               # dummy, dxbf, sqbf
    SQ_RINV2 = 5
    SQ_POFFBF = 6
    SQ_COPY = 7

    with nc.Block() as block:

        @block.sync
        def _(sync):
            sync.dma_start(rji[:, 0:80], rji_d[:, 0:80]).then_inc(dsem, 16)
            sync.wait_ge(sqm, SQ_COPY)
            sync.wait_ge(vq, VQ_CPB)
            sync.dma_start(
                out_d,
                bass.AP(outsb.tensor, 0, [[32 * 432, 4], [1, 432]]),
            ).then_inc(dsem, 16)
            sync.wait_ge(dsem, 48)

        @block.tensor
        def _(tensor):
            tn = [0]

            def T(inst):
                # no intra-PE serialization: MMs are independent (disjoint
                # PSUM) and the PE is strict FIFO; keep only the counter
                inst.then_inc(mmq, 1)
                tn[0] += 1
                return inst

            tensor.wait_ge(gq, 1)
            T(tensor.matmul(psum_warm[0:1, 0:1], ones_bf, ones_bf))
            T(tensor.matmul(psum_warm[0:1, 1:2], ones_bf, ones_bf))
            assert tn[0] == MM_WARM
            # p-state keep-warm pings while the DVE streams geometry/fc
            tensor.wait_ge(vq, VQ_R2)
            tensor.matmul(psum_warm[0:1, 0:1], ones_bf, ones_bf)
            tensor.wait_ge(vq, 13)
            tensor.matmul(psum_warm[0:1, 1:2], ones_bf, ones_bf)
            tensor.wait_ge(vq, VQ_ECH)
            T(tensor.matmul(psum_out[0:1, :], ones_bf, wx[:, 2 * NI:11 * NI]))
            tensor.wait_ge(vq, VQ_S1P)
            T(tensor.matmul(psum_out[32:33, :], ones_bf, big[:, 0:9 * NI]))
            tensor.wait_ge(vq, VQ_S2DP)
            T(tensor.matmul(psum_out[64:65, :], ones_bf, big[:, 9 * NI:18 * NI]))
            assert tn[0] == MM_CPA
            tensor.wait_ge(vq, VQ_ALL)
            T(tensor.matmul(psum_out2[0:1, :], ones_bf, big[:, 18 * NI:27 * NI]))
            assert tn[0] == MM_ALL

        @block.scalar
        def _(scalar):
            sn = [0]

            def S(inst):
                if sn[0] > 0:
                    inst._wait_ge(sqm, sn[0])
                inst.then_inc(sqm, 1)
                sn[0] += 1
                return inst

            # second half of the input on the scalar HWDGE queue
            scalar.dma_start(rji[:, 80:160], rji_d[:, 80:160]).then_inc(dsem, 16)
            scalar.wait_ge(gq, GQ_EPS)
            S(scalar.activation(
                scra[0:1, 0:1], c_eps[0:1, :], ACT.Abs_reciprocal_sqrt,
                bias=R2_EPS))
            # bf16 staging copies for the 2x-mode products
            scalar.wait_ge(vq, VQ_DX)
            S(scalar.activation(geobf[:, 0:144], dx, ACT.Copy))
            scalar.wait_ge(vq, VQ_SQ)
            S(scalar.activation(geobf[:, 144:288], sq_t, ACT.Copy))
            scalar.wait_ge(vq, VQ_R2)
            S(scalar.activation(rvp[:, NI:2 * NI], r2, ACT.Abs_reciprocal_sqrt,
                                bias=R2_EPS))
            S(scalar.activation(rvp[:, 0:NI], rvp[:, NI:2 * NI], ACT.Square,
                                bias=R2_EPS))
            assert sn[0] == SQ_RINV2
            scalar.wait_ge(gq, GQ_POFF)
            S(scalar.activation(geobf[:, 288:432], poff, ACT.Copy))
            assert sn[0] == SQ_POFFBF
            scalar.wait_ge(mmq, MM_CPA)
            S(scalar.activation(outsb[0:65, :], psum_out, ACT.Copy))
            assert sn[0] == SQ_COPY

        @block.gpsimd
        def _(gpsimd):
            gn = [0]

            def G(inst):
                if gn[0] > 0:
                    inst._wait_ge(gq, gn[0])
                inst.then_inc(gq, 1)
                gn[0] += 1
                return inst

            G(gpsimd.memset(ones_bf, 1.0))
            G(gpsimd.memset(c_eps, R2_EPS))
            G(gpsimd.memset(rvp[:, 2 * NI:3 * NI], 1.0))
            gpsimd.wait_ge(vq, VQ_DX)
            G(gpsimd.tensor_tensor(
                poff[:, 0:96], dx[:, 0:96], dx[:, 48:144], op=ALU.mult))
            G(gpsimd.tensor_tensor(
                poff[:, 96:144], dx[:, 0:NI], dx[:, 96:144], op=ALU.mult))
            assert gn[0] == GQ_POFF
            # w^2 for the e-chain (into the freed wrap scratch)
            gpsimd.wait_ge(vq, VQ_W)
            G(gpsimd.tensor_tensor(
                scr[:, 0:NI], r2w[:, NI:2 * NI], r2w[:, NI:2 * NI],
                op=ALU.mult))
            assert gn[0] == GQ_W2

        @block.vector
        def _(vector):
            vn = [0]

            def V(inst):
                if vn[0] > 0:
                    inst._wait_ge(vq, vn[0])
                inst.then_inc(vq, 1)
                vn[0] += 1
                return inst

            vector.wait_ge(dsem, 32)
            V(vector.tensor_tensor(dxr3, rj3, ri3, op=ALU.subtract))
            # minimum image (box = BOX_L * I)
            V(vector.tensor_scalar(
                scr[:, 0:144], dxr, BOX_L / 2, BOX_L, op0=ALU.is_gt, op1=ALU.mult))
            V(vector.tensor_tensor(
                scr[:, 144:288], dxr, scr[:, 0:144], op=ALU.subtract))
            V(vector.tensor_scalar(
                scr[:, 0:144], dxr, -BOX_L / 2, BOX_L, op0=ALU.is_lt, op1=ALU.mult))
            V(vector.tensor_tensor(
                dx, scr[:, 144:288], scr[:, 0:144], op=ALU.add))
            assert vn[0] == VQ_DX
            V(vector.tensor_tensor(sq_t, dx, dx, op=ALU.mult))
            assert vn[0] == VQ_SQ
            V(vector.reduce_sum(
                r2, _v(geo, 144, [[1, NI], [NI, 3]]),
                axis=mybir.AxisListType.X,
            ))
            assert vn[0] == VQ_R2
            V(vector.tensor_scalar(w, r2, RC * RC, None, op0=ALU.min))
            assert vn[0] == VQ_W
            V(vector.tensor_scalar(yh, w, c[FC_DEG], None, op0=ALU.mult))
            for k in range(FC_DEG - 1, 0, -1):
                V(vector.scalar_tensor_tensor(
                    yh, yh, c[k], w, op0=ALU.add, op1=ALU.mult))
            # e-family (bf16): [e0|e1|e2] = (C0P+yh) * [rinv2|rinv|1]
            vector.wait_ge(sqm, SQ_RINV2)
            vector.wait_ge(gq, 3)
            V(vector.scalar_tensor_tensor(
                _v(wx, 0, [[NI, 3], [1, NI]]),
                _v(yh, 0, [[0, 3], [1, NI]]),
                C0P,
                rvp[:, 0:144],
                op0=ALU.add, op1=ALU.mult))
            # [e3|e4] = [e1|e2]*w ; [e5..e8] = [e1..e4]*w2 ; [e9|e10] = [e5|e6]*w2
            V(vector.tensor_tensor(
                _v(wx, 3 * NI, [[NI, 2], [1, NI]]),
                _v(wx, NI, [[NI, 2], [1, NI]]),
                _v(r2w, NI, [[0, 2], [1, NI]]),
                op=ALU.mult))
            vector.wait_ge(gq, GQ_W2)
            V(vector.tensor_tensor(
                _v(wx, 5 * NI, [[NI, 4], [1, NI]]),
                _v(wx, NI, [[NI, 4], [1, NI]]),
                _v(scr, 0, [[0, 4], [1, NI]]),
                op=ALU.mult))
            V(vector.tensor_tensor(
                _v(wx, 9 * NI, [[NI, 2], [1, NI]]),
                _v(wx, 5 * NI, [[NI, 2], [1, NI]]),
                _v(scr, 0, [[0, 2], [1, NI]]),
                op=ALU.mult))
            assert vn[0] == VQ_ECH
            # all-bf16 S1/S2 product blocks (2x DVE mode)
            vector.wait_ge(sqm, SQ_GEOBF)
            V(vector.tensor_tensor(
                _v(big, 0, [[3 * NI, 3], [NI, 3], [1, NI]]),
                _v(wx, NI, [[NI, 3], [0, 3], [1, NI]]),
                _v(geobf, 0, [[0, 3], [NI, 3], [1, NI]]),
                op=ALU.mult))
            assert vn[0] == VQ_S1P
            V(vector.tensor_tensor(
                _v(big, 9 * NI, [[3 * NI, 3], [NI, 3], [1, NI]]),
                _v(wx, 0, [[NI, 3], [0, 3], [1, NI]]),
                _v(geobf, 144, [[0, 3], [NI, 3], [1, NI]]),
                op=ALU.mult))
            assert vn[0] == VQ_S2DP
            vector.wait_ge(sqm, SQ_POFFBF)
            V(vector.tensor_tensor(
                _v(big, 18 * NI, [[3 * NI, 3], [NI, 3], [1, NI]]),
                _v(wx, 0, [[NI, 3], [0, 3], [1, NI]]),
                _v(geobf, 288, [[0, 3], [NI, 3], [1, NI]]),
                op=ALU.mult))
            assert vn[0] == VQ_ALL
            # evacuate the bank-B moment row cross-quadrant to partition 96
            vector.wait_ge(mmq, MM_ALL)
            V(vector.tensor_scalar(
                outsb[96:97, :], psum_out2, 1.0, None, op0=ALU.mult))
            assert vn[0] == VQ_CPB

    nc.compile()
    return nc


def host_prep(R):
    """Per-core input arrays: [96, 160] = [R_i-chunk^T replicated | R_j | pad]."""
    R = np.ascontiguousarray(R, np.float32)
    in_maps = []
    for core in range(NJB * NIC):
        jh, ic = divmod(core, NIC)
        rji = np.zeros((NJ, 160), np.float32)
        ri = R[ic * NI:(ic + 1) * NI, :]              # [48, 3] central chunk
        rji[:, 0:144] = ri.T.reshape(1, 144)          # d-major, replicated
        rji[:, 144:147] = R[jh * NJ:(jh + 1) * NJ, :]
        in_maps.append({"rji": rji})
    return in_maps


def host_combine(partials):
    """partials: list of 8 [4,432] arrays (core order jh*4+ic). [192,18]."""
    sums = np.zeros((N, 36), np.float64)
    for core, p in enumerate(partials):
        jh, ic = divmod(core, NIC)
        q = p.astype(np.float64).reshape(4, 9, NI)
        sl = slice(ic * NI, (ic + 1) * NI)
        sums[sl, 0:9] += q[0].T
        sums[sl, 9:18] += q[1].T
        sums[sl, 18:27] += q[2].T
        sums[sl, 27:36] += q[3].T
    sums = sums.astype(np.float32)
    q_r = sums[:, 0:9].copy()
    q_r[:, 0] -= C0SELF
    s0 = q_r[:, 0:3]
    s1 = sums[:, 9:18].reshape(N, 3, 3)
    s2d = sums[:, 18:27].reshape(N, 3, 3)
    s2o = sums[:, 27:36].reshape(N, 3, 3)
    ang = np.empty((N, 3, 3), np.float32)
    ang[:, :, 0] = s0 * s0
    ang[:, :, 1] = (s1 * s1).sum(-1)
    fro2 = (s2d * s2d).sum(-1) + 2.0 * (s2o * s2o).sum(-1)
    ang[:, :, 2] = 1.5 * fro2 - 0.5 * s0 * s0
    return np.concatenate([q_r, ang.reshape(N, 9)], axis=-1)


def _get_nc():
    if "nc" not in _cached:
        _cached["nc"] = build_nc()
    return _cached["nc"]


def _make_runner(nc, n_cores):
    import jax
    from jax.sharding import Mesh, PartitionSpec
    from concourse import bass2jax
    from concourse import mybir as _mb

    shard_map = bass2jax.shard_map

    bass2jax.install_neuronx_cc_hook()
    partition_name = (
        nc.partition_id_tensor.name if nc.partition_id_tensor else None
    )
    in_names, out_names, out_avals = [], [], []
    for alloc in nc.m.functions[0].allocations:
        if not isinstance(alloc, _mb.MemoryLocationSet):
            continue
        name = alloc.memorylocations[0].name
        if alloc.kind == "ExternalInput":
            if name != partition_name:
                in_names.append(name)
        elif alloc.kind == "ExternalOutput":
            out_names.append(name)
            out_avals.append(jax.core.ShapedArray(
                tuple(alloc.tensor_shape), _mb.dt.np(alloc.dtype)))
    n_params = len(in_names)
    all_names = in_names + out_names
    if partition_name is not None:
        all_names = all_names + [partition_name]
    all_names = tuple(all_names)

    def _body(*args):
        operands = list(args)
        if partition_name is not None:
            operands.append(bass2jax.partition_id_tensor())
        outs = bass2jax._bass_exec_p.bind(
            *operands,
            out_avals=tuple(out_avals),
            in_names=all_names,
            out_names=tuple(out_names),
            lowering_input_output_aliases=(),
            sim_require_finite=True,
            sim_require_nnan=True,
            nc=nc,
        )
        return tuple(outs)

    devices = jax.devices()[:n_cores]
    mesh = Mesh(np.asarray(devices), ("core",))
    n_outs = len(out_names)
    sharded = jax.jit(
        shard_map(
            _body, mesh=mesh,
            in_specs=(PartitionSpec("core"),) * (n_params + n_outs),
            out_specs=(PartitionSpec("core"),) * n_outs,
            check_rep=False,
        ),
        donate_argnums=tuple(range(n_params, n_params + n_outs)),
        keep_unused=True,
    )

    def run(in_maps):
        concat_in = [
            np.concatenate([np.asarray(m[name]) for m in in_maps], axis=0)
            for name in in_names
        ]
        concat_zeros = [
            np.zeros((n_cores * a.shape[0], *a.shape[1:]), a.dtype)
            for a in out_avals
        ]
        out_arrs = sharded(*concat_in, *concat_zeros)
        return [
            {
                name: np.asarray(out_arrs[i]).reshape(
                    n_cores, *out_avals[i].shape)[c]
                for i, name in enumerate(out_names)
            }
            for c in range(n_cores)
        ]

    return run


def _get_runner():
    if "runner" not in _cached:
        _cached["runner"] = _make_runner(_get_nc(), NJB * NIC)
    return _cached["runner"]


def kernel(R, box):
    R = np.asarray(R, np.float32)
    box = np.asarray(box, np.float32)
    assert R.shape == (N, 3)
    assert np.allclose(box, np.eye(3, dtype=np.float32) * BOX_L), (
        "kernel compiled for box = 20*I"
    )
    in_maps = host_prep(R)
    results = _get_runner()(in_maps)
    partials = [results[c]["out"] for c in range(NJB * NIC)]
    return host_combine(partials)
